# revision 14
# baseline (speedup 1.0000x reference)
"""Trainium2 Bass kernel for nn_Affine_Linear_Abla_Quat.

Reference computation (per batch b, point n, channel d):
    R = quat2matrix(J[b,n,d])            (3x3 rotation)
    RTX = R^T X;  a = R[:,0]*RTX0 + R[:,1]*RTX1;  b = R[:,1]*RTX0 - R[:,0]*RTX1
    c = R[:,2]*RTX2
    Y[b,n,f,i] = sum_d A[f,d] a[...,i] + B[f,d] b[...,i] + C[f,d] c[...,i]

Algebraic simplification (R is an exact rotation; g = third column of R):
    a = x - g (g.x),   b = g x x (cross),   c = g (g.x)
    => Y_i = A.x_i + B.(g x x)_i + (C-A).(g_i (g.x))
g for an unnormalized quaternion (x,y,z,w), s = |q|^2:
    g = ( 2(xz+yw), 2(yz-xw), zz+ww-xx-yy ) / s

Device layout (host prepares everything in the exact on-chip layout):
  * channel d -> (h = d//128, partition p = d%128); points split into 8
    supersteps of 512.  Every field tile is [128, (slots), 2, 512] fp16.
  * J slots are ordered (qx, qz, qy, qw) so the four quaternion products
    and the (q1,q2) squares-pair reduction are all contiguous-slice ops.
  * weights are host-transposed to [d, f] fp16 and kept stationary on the
    PE; the 512-point term tiles stream as the moving operand, writing
    [f, n] into one PSUM bank per (i, f_half).  The cross-product
    subtractions are absorbed into the matmul accumulation via a negated
    weight copy (WBN = -B^T), and the a-term via WC = (C-A)^T.

Sharding: data-parallel over batch B=8 -> one batch per NeuronCore.
"""

import numpy as np

import concourse.bass as bass
import concourse.tile as tile
from concourse import mybir
from concourse.bass_utils import run_bass_kernel_spmd

F16 = mybir.dt.float16
F32 = mybir.dt.float32

N_CORES = 8
NPTS = 4096          # points per core (batch dim sharded)
D = 256              # in channels
F = 256              # out channels
P = 128              # partitions
SN = 512             # points per super-step
NS = NPTS // SN      # super-steps
RSQRT2 = 0.7071067811865476

ADD = mybir.AluOpType.add
SUB = mybir.AluOpType.subtract
MUL = mybir.AluOpType.mult


def _act_raw(nc, out, in_, func, scale=1.0):
    """InstActivation without the wrapper's Reciprocal guard (the guard
    targets fp32-precision use; this kernel is fp16 internally)."""
    eng = nc.scalar
    ins = [eng.lower_ap(in_),
           mybir.ImmediateValue(dtype=mybir.dt.float32, value=0.0),
           mybir.ImmediateValue(dtype=mybir.dt.float32, value=scale),
           mybir.ImmediateValue(dtype=mybir.dt.float32, value=0.0)]
    return eng.add_instruction(mybir.InstActivation(
        name=nc.get_next_instruction_name(), func=func,
        ins=ins, outs=[eng.lower_ap(out)]))


def _split_multi_waits(nc):
    """This container's walrus rejects instructions carrying more than one
    sync wait. Hoist extra waits onto same-engine NoOps inserted directly
    before the offending instruction (semantically identical: all waits
    must hold before the instruction issues, and the NoOps are adjacent)."""
    ctr = 0
    for f in nc.m.functions:
        for bb in f.blocks:
            out = []
            for inst in bb.instructions:
                si = inst.sync_info
                if si is not None and si.on_wait and len(si.on_wait) > 1:
                    waits = list(si.on_wait)
                    for w in waits[:-1]:
                        nop = mybir.InstNoOp(
                            name=f"waitnop_{ctr}", ins=[], outs=[])
                        ctr += 1
                        nop.engine = inst.engine
                        nop.bass_nofuse = True
                        nop.sync_info = mybir.SyncInfo(
                            on_wait=[w], on_update=[])
                        out.append(nop)
                    si.on_wait.clear()
                    si.on_wait.append(waits[-1])
                out.append(inst)
            bb.instructions[:] = out


def build_kernel(fixup=True):
    nc = bass.Bass("TRN2", target_bir_lowering=False, debug=False)
    # inputs, already fp16 + device-layout on host (pure relayout/cast)
    j_d = nc.dram_tensor("JB", [NS, P, 4, 2, SN], F16, kind="ExternalInput").ap()
    x_d = nc.dram_tensor("XB", [NS, P, 3, 2, SN], F16, kind="ExternalInput").ap()
    wa_d = nc.dram_tensor("WA", [P, 2, F], F16, kind="ExternalInput").ap()
    wb_d = nc.dram_tensor("WB", [P, 2, F], F16, kind="ExternalInput").ap()
    wbn_d = nc.dram_tensor("WBN", [P, 2, F], F16, kind="ExternalInput").ap()
    wc_d = nc.dram_tensor("WC", [P, 2, F], F16, kind="ExternalInput").ap()
    y_d = nc.dram_tensor("YB", [NS, P, 2, 3, SN], F16,
                         kind="ExternalOutput").ap()

    with tile.TileContext(nc) as tc:
        _body(nc, tc, j_d, x_d, (wa_d, wb_d, wbn_d, wc_d), y_d)
    # non-self-loading matmuls: their sem waits must ride on the paired
    # LDWEIGHTS (the PE sequencer applies waits before the weight load)
    import concourse.bass_primitives_rust as _bpr
    try:
        _bpr.move_matmul_waits_to_ldweights(nc.m)
    except AttributeError:
        from concourse import bass as _b
        _b._bass_rust.move_matmul_waits_to_ldweights(nc.m)
    if fixup:
        _split_multi_waits(nc)
    return nc


def _body(nc, tc, j_d, x_d, w_d, y_d):
    from contextlib import ExitStack
    ctx = ExitStack()
    with ctx:
        singles = ctx.enter_context(tc.tile_pool(name="singles", bufs=1))
        inp = ctx.enter_context(tc.tile_pool(name="inp", bufs=2))
        sqp = ctx.enter_context(tc.tile_pool(name="sqp", bufs=2))
        mid = ctx.enter_context(tc.tile_pool(name="mid", bufs=2))
        term = ctx.enter_context(tc.tile_pool(name="term", bufs=2))
        ysbp = ctx.enter_context(tc.tile_pool(name="ysb", bufs=2))
        ypps = ctx.enter_context(tc.tile_pool(name="ypps", bufs=1, space="PSUM"))

        # preload the reciprocal_and_small ACT table set while the first
        # DMAs are in flight (Square/Copy/Reciprocal all live in this set)
        warm = singles.tile([P, 16], F16, tag="actwarm")
        nc.vector.memset(warm[:], 1.0)
        _act_raw(nc, warm[:], warm[:],
                 mybir.ActivationFunctionType.Reciprocal)

        # weights -> SBUF [128, 2, 256]: (d_local, d_half, f)
        wts = []
        for name, wd in zip(("wa", "wb", "wbn", "wc"), w_d):
            wt = singles.tile([P, 2, F], F16, tag=f"w_{name}")
            nc.sync.dma_start(wt[:], wd)
            wts.append(wt)
        wa, wb, wbn, wc = wts

        prev = None  # (ypall, ysb, s) pending evacuation
        for s in range(NS):
            # ---- loads (HWDGE on the SP ring; plain fp16, no casting) ----
            jt = inp.tile([P, 4, 2, SN], F16, tag="jt")   # qx, qz, qy, qw
            nc.sync.dma_start(jt[:], j_d[s])
            xt = inp.tile([P, 3, 2, SN], F16, tag="xt")   # x0, x1, x2
            nc.sync.dma_start(xt[:], x_d[s])

            # ---- half-squares of the quaternion (ACT) ----
            # sq slots follow jt order: (xx, zz, ww, yy) / 2
            sq = sqp.tile([P, 4, 2, SN], F16, tag="sq")
            nc.scalar.activation(sq[:], jt[:],
                                 mybir.ActivationFunctionType.Square,
                                 scale=RSQRT2)

            # ---- DVE chain ----
            def tt(pool, name, a, b, op, shape=(2, SN)):
                t = pool.tile([P, *shape], F16, tag=name)
                nc.vector.tensor_tensor(out=t[:], in0=a, in1=b, op=op)
                return t

            def bc(ap, k):  # repeat a [P,2,SN] field k ways (0-stride dim)
                return ap.unsqueeze(1).broadcast_to([P, k, 2, SN])

            # sh = s/2 first so ACT's recip can issue while the rest of the
            # quaternion chain runs.
            q1 = tt(mid, "q1", sq[:, 0], sq[:, 3], ADD)       # (xx+yy)/2
            q2 = tt(mid, "q2", sq[:, 1], sq[:, 2], ADD)       # (zz+ww)/2
            sh = tt(mid, "sh", q1[:], q2[:], ADD)
            invh = mid.tile([P, 2, SN], F16, tag="invh")
            _act_raw(nc, invh[:], sh[:],
                     mybir.ActivationFunctionType.Reciprocal)  # 2/s

            vt = mid.tile([P, 3, 2, SN], F16, tag="vt")       # v/2
            nc.vector.tensor_tensor(out=vt[:, 2], in0=q2[:],
                                    in1=q1[:], op=SUB)
            # (xz, xw) and (yz, yw) as broadcast-pair products
            p1 = tt(mid, "p1", bc(jt[:, 0], 2), jt[:, 1:3], MUL, (2, 2, SN))
            p2 = tt(mid, "p2", bc(jt[:, 3], 2), jt[:, 1:3], MUL, (2, 2, SN))
            nc.vector.tensor_tensor(out=vt[:, 0], in0=p1[:, 0],
                                    in1=p2[:, 1], op=ADD)     # xz + yw
            nc.vector.tensor_tensor(out=vt[:, 1], in0=p2[:, 0],
                                    in1=p1[:, 1], op=SUB)     # yz - xw

            g = mid.tile([P, 3, 2, SN], F16, tag="g")
            nc.vector.tensor_tensor(out=g[:], in0=vt[:],
                                    in1=bc(invh[:], 3), op=MUL)
            pd = tt(mid, "pd", g[:, 0:3], xt[:, 0:3], MUL, (3, 2, SN))
            dota = tt(mid, "dota", pd[:, 0], pd[:, 1], ADD)
            dot = tt(mid, "dot", dota[:], pd[:, 2], ADD)
            ct = term.tile([P, 3, 2, SN], F16, tag="ct")      # g_i (g.x)
            nc.vector.tensor_tensor(out=ct[:], in0=g[:],
                                    in1=bc(dot[:], 3), op=MUL)
            # cross products: slots (p12, p20, p01, p21, p02, p10)
            # cross_i = p_{i+1,i+2} - p_{i+2,i+1}; the minus rides on WBN.
            cr = term.tile([P, 6, 2, SN], F16, tag="cr")
            for sl, (a, b) in enumerate(
                    ((1, 2), (2, 0), (0, 1), (2, 1), (0, 2), (1, 0))):
                nc.vector.tensor_tensor(out=cr[:, sl], in0=g[:, a],
                                        in1=xt[:, b], op=MUL)

            # ---- channel-mix matmuls: weights stationary, terms moving ----
            # separate PSUM tiles per f-half so the next super's A-block only
            # waits on that half's evacuation (sub-generation WAR tracking)
            yp0 = ypps.tile([P, 3, SN], F32, tag="yp0")   # fh=0, slot=i
            yp1 = ypps.tile([P, 3, SN], F32, tag="yp1")   # fh=1, slot=i
            yps = (yp0, yp1)
            pieces = [(wa, [xt[:, 0], xt[:, 1], xt[:, 2]]),
                      (wc, [ct[:, 0], ct[:, 1], ct[:, 2]]),
                      (wb, [cr[:, 0], cr[:, 1], cr[:, 2]]),
                      (wbn, [cr[:, 3], cr[:, 4], cr[:, 5]])]

            def mm_block(piece_ids, fhs):
                for pi in piece_ids:
                    wt, terms = pieces[pi]
                    for fh in fhs:
                        for h in range(2):
                            # load the stationary once; the three i-matmuls
                            # reuse it (non-self-loading), so their fills
                            # pipeline back-to-back instead of serializing
                            # on a weight reload + array drain each time.
                            piece = wt[:, h, fh * P:(fh + 1) * P]
                            nc.tensor.ldweights(piece)
                            for i in range(3):
                                mi = nc.tensor.matmul(
                                    yps[fh][:, i, :],
                                    lhsT=piece,
                                    rhs=terms[i][:, h, :],
                                    start=(pi == 0 and h == 0),
                                    stop=(pi == 3 and h == 1))
                                mi.ins.ldweights = False

            # A and C' fill the PE while the DVE finishes the cross products;
            # B/BN run per f-half so each half's banks free up early.
            mm_block([0], (0, 1))        # A (x terms)
            mm_block([1], (0, 1))        # C-A (c terms)
            mm_block([2, 3], (0,))       # B, -B  fh0
            mm_block([2, 3], (1,))       # B, -B  fh1

            # ---- evacuate the PREVIOUS super's PSUM (ACT has slack here;
            # emitting it after this super's recip keeps the DVE-critical
            # reciprocal from queueing behind PE-dependent copies) ----
            if prev is not None:
                _evac(nc, prev, y_d)
            ysb = ysbp.tile([P, 2, 3, SN], F16, tag="ysb")
            prev = ((yp0, yp1), ysb, s)
        _evac(nc, prev, y_d)


def _evac(nc, prev, y_d):
    (yp0, yp1), ysb, s = prev
    nc.scalar.copy(ysb[:, 0], yp0[:])   # one FD-1536 copy per f-half
    nc.scalar.copy(ysb[:, 1], yp1[:])
    nc.scalar.dma_start(y_d[s], ysb[:])


_BUILT = {}

# test-harness hooks (ignored in normal use)
TRACE = False
LAST_EXEC_NS = None
LAST_RESULT = None


def _get_nc():
    if "nc" not in _BUILT:
        _BUILT["nc"] = build_kernel()
    return _BUILT["nc"]


def _pack_inputs(X, J, A, B, C):
    def packw(M):
        return np.ascontiguousarray(
            M.T.astype(np.float16).reshape(2, P, F).transpose(1, 0, 2))

    wa, wb, wbn, wc = packw(A), packw(B), packw(-B), packw(C - A)

    in_maps = []
    for b in range(N_CORES):
        # [n, d, c] -> [s, p, c, h, nn]
        xb = X[b].astype(np.float16).reshape(NS, SN, 2, P, 3)
        xb = np.ascontiguousarray(xb.transpose(0, 3, 4, 2, 1))
        jb = J[b].astype(np.float16).reshape(NS, SN, 2, P, 4)
        jb = np.ascontiguousarray(
            jb.transpose(0, 3, 4, 2, 1)[:, :, [0, 2, 3, 1]])
        in_maps.append({
            "XB": xb, "JB": jb,
            "WA": wa, "WB": wb, "WBN": wbn, "WC": wc,
        })
    return in_maps


def kernel(X, J, A, B, C):
    """X [8,4096,256,3] f32, J [8,4096,256,4] f32, A/B/C [256,256] f32
    -> Y [8,4096,256,3] f32."""
    X = np.asarray(X)
    J = np.asarray(J)
    A = np.asarray(A, dtype=np.float32)
    B = np.asarray(B, dtype=np.float32)
    C = np.asarray(C, dtype=np.float32)

    nc = _get_nc()
    in_maps = _pack_inputs(X, J, A, B, C)
    global LAST_EXEC_NS, LAST_RESULT
    res = run_bass_kernel_spmd(nc, in_maps, core_ids=list(range(N_CORES)),
                               trace=TRACE)
    LAST_EXEC_NS = res.exec_time_ns
    LAST_RESULT = res
    # device YB [s, p(f_local), fh, i, nn] -> [n, f, i] fp32
    out = np.empty((N_CORES, NPTS, F, 3), dtype=np.float32)
    for b in range(N_CORES):
        yb = res.results[b]["YB"].reshape(NS, P, 2, 3, SN).astype(np.float32)
        out[b] = yb.transpose(0, 4, 2, 1, 3).reshape(NPTS, F, 3)
    return np.ascontiguousarray(out)
